# revision 17
# baseline (speedup 1.0000x reference)
"""Canny edge detector (nn_CannyNet) on 8 Trainium2 NeuronCores.

Self-contained: hardcodes shapes [4,3,1024,1024] and the filter constants.

Decomposition: 27 units = (3 channels) x (9 row-blocks: 8x120 + 1x64 rows).
Uniform SPMD program: every core processes 4 units (5 cores repeat their
first unit; host ignores the duplicate). Per unit, the 4 batch planes of one
channel are processed together because the reference's flat NMS gather
couples batches.

Engine split (per plane, [<=128, 1024] tiles):
  Pool : s1/s2 col shift-adds, u=v+s1, H=u+w, m2=sqx+sqy (plain TT adds only)
  DVE  : v=s2*R0, w=x*R2 (fast TS), masks + fp16 NMS compare cascade
  PE   : gx/gy directly via accumulating banded matmuls with col-shifted
         rhs views of the zero-padded H tile (vertical conv x horizontal taps)
  Act  : squares (with tan-scale folding) + signs straight from PSUM, fp16 out
All NMS compares run in fp16 (2x DVE rate), planes packed in pairs (1,3)/(0,2)
so one compare instruction covers both planes of an AND-candidate.
Row-shifted m2 views (m2p/m2m) via SBUF->SBUF DMA (engines cannot shift
partitions). Final out is fp16 0/1, converted on host.
"""
import math
import os
import numpy as np

import concourse.bass as bass
import concourse.mybir as mybir
from concourse.bass_utils import run_bass_kernel_spmd

ALU = mybir.AluOpType
AF = mybir.ActivationFunctionType
DT = mybir.dt.float32
F16 = mybir.dt.float16
U16 = mybir.dt.uint16

B, C, H_IMG, W = 4, 3, 1024, 1024
NU = 1 if os.environ.get('KDBG') else 4  # units per core (uniform)
M = 122           # m2/out row span per unit (out 120 + 2)
XR = 128          # x-tile rows
FW = 1028         # x-tile width (cols -2..1025)
HW2 = 1026        # H tile width (cols -1..1024, zero side cols)
MW = 1026         # m2 width per plane (cols -1..1024)

_g = np.exp(-0.5 * np.arange(-2, 3, dtype=np.float64) ** 2)
G1 = _g[1]
R0 = float(np.float32(_g[0] / _g[1]))   # g0/g1
R2 = float(np.float32(1.0 / _g[1]))     # 1/g1
THR = float(np.float32((400.0 / (127.5 * G1)) ** 2))
_t1 = math.tan(22.5 * 3.14159 / 180.0)
_t3 = math.tan(67.5 * 3.14159 / 180.0)

# units and core assignment
UNITS = [(c, k) for k in range(9) for c in range(3)]  # 27
CORE_UNITS = []
for i in range(8):
    us = [UNITS[i], UNITS[i + 8], UNITS[i + 16]]
    us.append(UNITS[24 + i] if i < 3 else UNITS[i])  # dummy repeat for cores 3..7
    CORE_UNITS.append(us)


def _unit_rows(k):
    """(xbase, out0): x-tile img rows xbase..xbase+127; out rows out0..out0+119
    (k=8: only first 64 valid)."""
    if k < 8:
        return 120 * k - 4, 120 * k
    return 900, 960


def _make_bands():
    """CX = S121 @ G, CY = S101 @ G over image rows with zero-pad truncation."""
    n = H_IMG
    G = np.zeros((n, n), np.float64)
    for kk in range(-2, 3):
        v = _g[kk + 2]
        for o in range(max(0, -kk), min(n, n - kk)):
            G[o, o + kk] = v
    S121 = np.zeros((n, n), np.float64)
    S101 = np.zeros((n, n), np.float64)
    for o in range(n):
        for kk, w1, w2 in ((-1, 1.0, 1.0), (0, 2.0, 0.0), (1, 1.0, -1.0)):
            i = o + kk
            if 0 <= i < n:
                S121[o, i] = w1
                if kk != 0:
                    S101[o, i] = w2
    CX = (S121 @ G).astype(np.float32)
    CY = (S101 @ G).astype(np.float32)
    return CX, CY


def _band_lhsT(Cm, k):
    """lhsT [XR, M]: lhsT[kr, m] = Cm[out0-1+m, xbase+kr] (0 out of range)."""
    xbase, out0 = _unit_rows(k)
    out = np.zeros((XR, M), np.float32)
    for m in range(M):
        orow = out0 - 1 + m
        if not (0 <= orow < H_IMG):
            continue
        for d in range(-3, 4):
            irow = orow + d
            kr = irow - xbase
            if 0 <= irow < H_IMG and 0 <= kr < XR:
                out[kr, m] = Cm[orow, irow]
    return out


def build_nc():
    nc = bass.Bass()
    xin = nc.declare_dram_parameter("xin", [NU, B, XR, FW], DT, isOutput=False)
    bands = nc.declare_dram_parameter("bands", [128, NU * 4 * M], DT, isOutput=False)
    outd = nc.declare_dram_parameter("out", [NU, B, 120, W], F16, isOutput=True)
    dbg = None
    if os.environ.get('KDBG'):
        dbg = {
            "dbg_t02": nc.declare_dram_parameter("dbg_t02", [M, 2, MW], DT, isOutput=True),
            "dbg_t13": nc.declare_dram_parameter("dbg_t13", [M, 2, MW], DT, isOutput=True),
            "dbg_mhv": nc.declare_dram_parameter("dbg_mhv", [B, M, 2 * W], U16, isOutput=True),
            "dbg_d1m": nc.declare_dram_parameter("dbg_d1m", [B, M, W], U16, isOutput=True),
            "dbg_h": nc.declare_dram_parameter("dbg_h", [2, XR, HW2], DT, isOutput=True),
            "dbg_sq": nc.declare_dram_parameter("dbg_sq", [2, M, 3 * W], F16, isOutput=True),
        }

    from contextlib import ExitStack
    es = ExitStack()
    ent = es.enter_context

    x = [ent(nc.sbuf_tensor(f"x{b}", [XR, FW], DT)) for b in range(B)]
    s1 = [ent(nc.sbuf_tensor(f"s1_{j}", [XR, W], DT)) for j in range(2)]
    s2 = [ent(nc.sbuf_tensor(f"s2_{j}", [XR, W], DT)) for j in range(2)]
    vv = [ent(nc.sbuf_tensor(f"vv{j}", [XR, W], DT)) for j in range(2)]
    ww = [ent(nc.sbuf_tensor(f"ww{j}", [XR, W], DT)) for j in range(2)]
    Ht = [ent(nc.sbuf_tensor(f"Ht{j}", [XR, HW2], DT)) for j in range(2)]
    ut = s2  # ut aliases s2: s2[j] is dead once DVE v(b) = s2*R0 has read it
    # fp16 working set
    sqa = [ent(nc.sbuf_tensor(f"sqa{j}", [M, 3 * W], F16)) for j in range(2)]
    tsqx = [ent(nc.sbuf_tensor(f"tsqx{j}", [M, W], DT)) for j in range(2)]
    tsqy = [ent(nc.sbuf_tensor(f"tsqy{j}", [M, W], DT)) for j in range(2)]
    sgx = [ent(nc.sbuf_tensor(f"sgx{j}", [M, W], F16)) for j in range(2)]
    sgy = [ent(nc.sbuf_tensor(f"sgy{j}", [M, W], F16)) for j in range(2)]
    mhv = [ent(nc.sbuf_tensor(f"mhv{b}", [M, 2 * W], U16)) for b in range(B)]
    d1m = [ent(nc.sbuf_tensor(f"d1m{b}", [M, W], U16)) for b in range(B)]
    # packed m2 tiles: [M, 2, MW] — slice 0/1 = planes (0,2) in t02, (1,3) in t13
    t02 = ent(nc.sbuf_tensor("t02", [M, 2, MW], DT))
    t13 = ent(nc.sbuf_tensor("t13", [M, 2, MW], DT))
    t02p = ent(nc.sbuf_tensor("t02p", [M, 2, MW], DT))
    t02m = ent(nc.sbuf_tensor("t02m", [M, 2, MW], DT))
    t13p = ent(nc.sbuf_tensor("t13p", [M, 2, MW], DT))
    t13m = ent(nc.sbuf_tensor("t13m", [M, 2, MW], DT))
    cn13 = ent(nc.sbuf_tensor("cn13", [M, 2, W], F16))
    cn02 = ent(nc.sbuf_tensor("cn02", [M, 2, W], F16))
    cp13 = ent(nc.sbuf_tensor("cp13", [M, 2, W], F16))
    cp02 = ent(nc.sbuf_tensor("cp02", [M, 2, W], F16))
    s12 = ent(nc.sbuf_tensor("s12", [M, W], F16))
    aD = ent(nc.sbuf_tensor("aD", [M, W], F16))
    aV = ent(nc.sbuf_tensor("aV", [M, W], F16))
    aH = ent(nc.sbuf_tensor("aH", [M, W], F16))
    thr_t = ent(nc.sbuf_tensor("thr_t", [M, W], F16))
    out_t = [ent(nc.sbuf_tensor(f"out_t{j}", [M, W], F16)) for j in range(2)]
    bnd = ent(nc.sbuf_tensor("bnd", [128, NU * 4 * M], DT))
    pgx = [ent(nc.psum_tensor(f"pgx{j}", [M, W], DT)) for j in range(2)]
    pgy = [ent(nc.psum_tensor(f"pgy{j}", [M, W], DT)) for j in range(2)]

    d_b = ent(nc.semaphore("d_b"))
    d_x = ent(nc.semaphore("d_x"))
    d_sh = ent(nc.semaphore("d_sh"))
    d_out = ent(nc.semaphore("d_out"))
    g_s = ent(nc.semaphore("g_s"))
    g_u = ent(nc.semaphore("g_u"))
    g_h = ent(nc.semaphore("g_h"))
    g_m2 = ent(nc.semaphore("g_m2"))
    v_v = ent(nc.semaphore("v_v"))
    v_w = ent(nc.semaphore("v_w"))
    v_mhv = ent(nc.semaphore("v_mhv"))
    v_d1 = ent(nc.semaphore("v_d1"))
    v_fin = ent(nc.semaphore("v_fin"))
    v_cmp = ent(nc.semaphore("v_cmp"))
    a_sq = ent(nc.semaphore("a_sq"))
    a_sg = ent(nc.semaphore("a_sg"))
    pe = ent(nc.semaphore("pe"))
    block = ent(nc.Block())

    def IX(u, b):
        return 4 * u + b + 1  # 1-based cumulative count at completion of (u,b)

    @block.sync
    def _(sync):
        sync.dma_start(out=bnd[:], in_=bands[:]).then_inc(d_b, 16)
        for b in range(B):
            sync.dma_start(out=x[b][:], in_=xin[0, b]).then_inc(d_x, 16)
        for u in range(NU):
            # x loads for u+1 BEFORE shifts/outs of u: DVE phaseC(u) comes
            # after phaseA(u+1), which needs Pool phase1(u+1) <- these loads.
            if u + 1 < NU:
                for b in range(B):
                    # x[b] WAR: Pool s2 and DVE w of (u,b) read x[b]
                    sync.wait_ge(g_s, IX(u, b))
                    sync.wait_ge(v_w, IX(u, b))
                    sync.dma_start(out=x[b][:], in_=xin[u + 1, b]).then_inc(d_x, 16)
            # per-plane m2 row shifts, launched as soon as each slice lands
            if u > 0:
                sync.wait_ge(v_cmp, 4 * u)  # WAR: NMS cmps of u-1 read shift tiles
            for b in range(B):
                sync.wait_ge(g_m2, IX(u, b))
                src = t02 if b % 2 == 0 else t13
                tp = t02p if b % 2 == 0 else t13p
                tm = t02m if b % 2 == 0 else t13m
                q = b // 2
                sync.dma_start(out=tp[0:M - 1, q], in_=src[1:M, q]).then_inc(d_sh, 16)
                sync.dma_start(out=tm[1:M, q], in_=src[0:M - 1, q]).then_inc(d_sh, 16)
            for b in range(B):
                sync.wait_ge(v_fin, IX(u, b))
                sync.dma_start(out=outd[u, b],
                               in_=out_t[(4 * u + b) % 2][1:121, :]).then_inc(d_out, 16)
        ndbg = 0
        if dbg is not None:
            sync.wait_ge(v_fin, NU * B)
            sync.dma_start(out=dbg["dbg_t02"][:], in_=t02[:]).then_inc(d_out, 16)
            sync.dma_start(out=dbg["dbg_t13"][:], in_=t13[:]).then_inc(d_out, 16)
            for b in range(B):
                sync.dma_start(out=dbg["dbg_mhv"][b], in_=mhv[b][:]).then_inc(d_out, 16)
                sync.dma_start(out=dbg["dbg_d1m"][b], in_=d1m[b][:]).then_inc(d_out, 16)
            for j in range(2):
                sync.dma_start(out=dbg["dbg_h"][j], in_=Ht[j][:]).then_inc(d_out, 16)
                sync.dma_start(out=dbg["dbg_sq"][j], in_=sqa[j][:]).then_inc(d_out, 16)
            ndbg = 14
        sync.wait_ge(d_out, 16 * (NU * B + ndbg))

    @block.gpsimd
    def _(gpsimd):
        # prologue: zero pads once — H side cols; m2 pad cols; shift edge rows
        for j in range(2):
            gpsimd.memset(Ht[j][:, 0:1], 0.0)
            gpsimd.memset(Ht[j][:, 1025:1026], 0.0)
        for t in (t02, t13):
            gpsimd.memset(t[:, :, 0:1], 0.0)
            gpsimd.memset(t[:, :, 1025:1026], 0.0)
        for t in (t02p, t02m, t13p, t13m):
            gpsimd.memset(t[:, :, 0:1], 0.0)
            gpsimd.memset(t[:, :, 1025:1026], 0.0)
        def ph12(u):
            # returns (s1s2, uh) closures; caller interleaves them
            def s1s2(b):
                ix = IX(u, b)
                gpsimd.wait_ge(d_x, 16 * ix)
                if 4 * u + b >= 2:
                    gpsimd.wait_ge(v_v, ix - 2)  # s2 slot WAR (DVE v read)
                gpsimd.tensor_tensor(out=s1[b % 2][:], in0=x[b][:, 1:1025],
                                     in1=x[b][:, 3:1027], op=ALU.add)
                gpsimd.tensor_tensor(out=s2[b % 2][:], in0=x[b][:, 0:1024],
                                     in1=x[b][:, 4:1028],
                                     op=ALU.add).then_inc(g_s, 1)

            def uh(b):
                ix = IX(u, b)
                gpsimd.wait_ge(v_v, ix)
                gpsimd.tensor_tensor(out=ut[b % 2][:], in0=vv[b % 2][:],
                                     in1=s1[b % 2][:], op=ALU.add).then_inc(g_u, 1)
                gpsimd.wait_ge(v_w, ix)
                if 4 * u + b >= 2:
                    gpsimd.wait_ge(pe, ix - 2)  # Ht slot WAR (PE read)
                gpsimd.tensor_tensor(out=Ht[b % 2][:, 1:1025], in0=ut[b % 2][:],
                                     in1=ww[b % 2][:], op=ALU.add).then_inc(g_h, 1)

            return s1s2, uh

        def m2one(u, b):
            ix = IX(u, b)
            gpsimd.wait_ge(a_sq, ix)
            if u > 0 and b == 0:
                gpsimd.wait_ge(v_cmp, 4 * u)   # m2 tiles WAR (NMS cmps of u-1)
                gpsimd.wait_ge(d_sh, 128 * u)  # and shift DMAs of u-1
            tile = t02 if b % 2 == 0 else t13
            q = b // 2
            gpsimd.tensor_tensor(out=tile[:, q, 1:1025], in0=tsqx[b % 2][:],
                                 in1=tsqy[b % 2][:],
                                 op=ALU.add).then_inc(g_m2, 1)

        a0, b0 = ph12(0)
        a0(0); a0(1); b0(0); a0(2); b0(1); a0(3); b0(2); b0(3)
        for u in range(NU):
            if u + 1 < NU:
                sn, un = ph12(u + 1)
                sn(0); sn(1)
                m2one(u, 0); m2one(u, 1)
                un(0); sn(2); un(1)
                m2one(u, 2)
                sn(3); un(2)
                m2one(u, 3)
                un(3)
            else:
                for b in range(B):
                    m2one(u, b)

    @block.vector
    def _(vector):
        def phaseB(u):
            for b in range(B):
                ix = IX(u, b)
                vector.wait_ge(a_sq, ix)
                vector.tensor_tensor(out=mhv[b][:], in0=sqa[b % 2][:, 0:2048],
                                     in1=sqa[b % 2][:, 1024:3072],
                                     op=ALU.is_ge).then_inc(v_mhv, 1)
                vector.wait_ge(a_sg, ix)
                vector.tensor_tensor(out=d1m[b][:], in0=sgx[b % 2][:],
                                     in1=sgy[b % 2][:],
                                     op=ALU.is_equal).then_inc(v_d1, 1)

        def phaseC(u):
            # b=0 (E/W) needs no row-shift tiles: only all m2 slices written
            vector.wait_ge(g_m2, 4 * (u + 1))
            for b in range(B):
                ix = IX(u, b)
                if b == 1:
                    vector.wait_ge(d_sh, 32 * B * (u + 1))
                if b == 0:
                    pv, mv_ = (t02[:, :, 2:1026], t13[:, :, 2:1026]), \
                              (t02[:, :, 0:1024], t13[:, :, 0:1024])
                elif b == 1:
                    pv, mv_ = (t02p[:, :, 2:1026], t13p[:, :, 2:1026]), \
                              (t02m[:, :, 0:1024], t13m[:, :, 0:1024])
                elif b == 2:
                    pv, mv_ = (t02p[:, :, 1:1025], t13p[:, :, 1:1025]), \
                              (t02m[:, :, 1:1025], t13m[:, :, 1:1025])
                else:
                    pv, mv_ = (t02p[:, :, 0:1024], t13p[:, :, 0:1024]), \
                              (t02m[:, :, 2:1026], t13m[:, :, 2:1026])
                c02 = t02[:, :, 1:1025]
                c13 = t13[:, :, 1:1025]
                vector.tensor_tensor(out=cn13[:], in0=c13, in1=mv_[1], op=ALU.is_gt)
                vector.tensor_tensor(out=cn02[:], in0=c02, in1=mv_[0], op=ALU.is_gt)
                vector.tensor_tensor(out=cp13[:], in0=c13, in1=pv[1], op=ALU.is_gt)
                vector.tensor_tensor(out=cp02[:], in0=c02, in1=pv[0], op=ALU.is_gt)
                tile = t02 if b % 2 == 0 else t13
                q = b // 2
                vector.tensor_scalar(out=thr_t[:], in0=tile[:, q, 1:1025],
                                     scalar1=THR, scalar2=None,
                                     op0=ALU.is_ge).then_inc(v_cmp, 1)
                # candidates: default=(c1>m1)&(c3>m3); d1=(c0>m0)&(c2>m2);
                #             mv=(c1>p1)&(c3>p3);      mh=(c0>p0)&(c2>p2)
                vector.tensor_tensor(out=s12[:], in0=cn13[:, 0, :],
                                     in1=cn13[:, 1, :], op=ALU.mult)
                vector.tensor_tensor(out=aD[:], in0=cn02[:, 0, :],
                                     in1=cn02[:, 1, :], op=ALU.mult)
                vector.tensor_tensor(out=aV[:], in0=cp13[:, 0, :],
                                     in1=cp13[:, 1, :], op=ALU.mult)
                vector.tensor_tensor(out=aH[:], in0=cp02[:, 0, :],
                                     in1=cp02[:, 1, :], op=ALU.mult)
                vector.copy_predicated(out=s12[:], mask=d1m[b][:], data=aD[:])
                vector.copy_predicated(out=s12[:], mask=mhv[b][:, 1024:2048],
                                       data=aV[:])
                vector.copy_predicated(out=s12[:], mask=mhv[b][:, 0:1024],
                                       data=aH[:])
                if 4 * u + b >= 2:
                    vector.wait_ge(d_out, 16 * (ix - 2))
                vector.tensor_tensor(out=out_t[(4 * u + b) % 2][:], in0=thr_t[:],
                                     in1=s12[:], op=ALU.mult).then_inc(v_fin, 1)

        for u in range(NU):
            phaseB(u)
            phaseC(u)

    @block.scalar
    def _(scalar):
        def vw(u, b):
            ix = IX(u, b)
            j = b % 2
            scalar.wait_ge(d_x, 16 * ix)
            if 4 * u + b >= 2:
                scalar.wait_ge(g_h, ix - 2)  # ww slot WAR (Pool H read)
            nc.scalar.activation(out=ww[j][:], in_=x[b][:, 2:1026],
                                 func=AF.Copy, scale=R2).then_inc(v_w, 1)
            scalar.wait_ge(g_s, ix)
            if 4 * u + b >= 2:
                scalar.wait_ge(g_u, ix - 2)  # vv slot WAR (Pool ut read)
            nc.scalar.activation(out=vv[j][:], in_=s2[j][:],
                                 func=AF.Copy, scale=R0).then_inc(v_v, 1)

        for b in range(B):
            vw(0, b)
        for u in range(NU):
            for b in range(B):
                if u + 1 < NU:
                    vw(u + 1, b)
                ix = IX(u, b)
                scalar.wait_ge(pe, ix)
                if 4 * u + b >= 2:
                    scalar.wait_ge(v_mhv, ix - 2)
                    scalar.wait_ge(g_m2, ix - 2)
                    scalar.wait_ge(v_d1, ix - 2)
                j = b % 2
                p, q = pgx[j], pgy[j]
                nc.scalar.activation(out=sqa[j][:, 0:1024], in_=p[:],
                                     func=AF.Square, scale=_t1)
                nc.scalar.activation(out=sqa[j][:, 1024:2048], in_=q[:],
                                     func=AF.Square)
                nc.scalar.activation(out=sqa[j][:, 2048:3072], in_=p[:],
                                     func=AF.Square, scale=_t3)
                nc.scalar.activation(out=tsqx[j][:], in_=p[:],
                                     func=AF.Square)
                nc.scalar.activation(out=tsqy[j][:], in_=q[:],
                                     func=AF.Square).then_inc(a_sq, 1)
                nc.scalar.activation(out=sgx[j][:], in_=p[:], func=AF.Sign)
                nc.scalar.activation(out=sgy[j][:], in_=q[:],
                                     func=AF.Sign).then_inc(a_sg, 1)

    @block.tensor
    def _(tensor):
        tensor.wait_ge(d_b, 16)
        for u in range(NU):
            off = u * 4 * M
            bxp = bnd[:, off + 0 * M:off + 1 * M]
            bxn = bnd[:, off + 1 * M:off + 2 * M]
            by1 = bnd[:, off + 2 * M:off + 3 * M]
            by2 = bnd[:, off + 3 * M:off + 4 * M]
            for b in range(B):
                ix = IX(u, b)
                tensor.wait_ge(g_h, ix)
                if 4 * u + b >= 2:
                    tensor.wait_ge(a_sg, ix - 2)  # PSUM pair WAR (Act reads)
                j = b % 2
                p, q, H_ = pgx[j], pgy[j], Ht[j]
                mm = nc.tensor.matmul
                mm(p[:, 0:512], bxp, H_[:, 0:512], start=True, stop=False)
                mm(p[:, 512:1024], bxp, H_[:, 512:1024], start=True, stop=False)
                mm(p[:, 0:512], bxn, H_[:, 2:514], start=False, stop=True)
                mm(p[:, 512:1024], bxn, H_[:, 514:1026], start=False, stop=True)
                mm(q[:, 0:512], by1, H_[:, 0:512], start=True, stop=False)
                mm(q[:, 512:1024], by1, H_[:, 512:1024], start=True, stop=False)
                mm(q[:, 0:512], by1, H_[:, 2:514], start=False, stop=False)
                mm(q[:, 512:1024], by1, H_[:, 514:1026], start=False, stop=False)
                mm(q[:, 0:512], by2, H_[:, 1:513], start=False, stop=True)
                mm(q[:, 512:1024], by2, H_[:, 513:1025], start=False,
                   stop=True).then_inc(pe, 1)

    es.close()
    return nc


_NC_CACHE = {}


def kernel(img, gauss_h=None, gauss_v=None, sobel_h=None, sobel_v=None,
           dir_w=None, **_):
    img = np.asarray(img, dtype=np.float32)
    assert img.shape == (B, C, H_IMG, W)

    # host pad with 0 and fold the (+1) affine in
    pad = np.zeros((B, C, H_IMG + 8, W + 4), np.float32)
    pad[:, :, 4:4 + H_IMG, 2:2 + W] = img + 1.0

    CX, CY = _make_bands()
    band_cache = {}
    for c, k in UNITS:
        if k not in band_cache:
            bx = _band_lhsT(CX, k)
            by = _band_lhsT(CY, k)
            band_cache[k] = (bx, -bx, by, 2.0 * by)

    in_maps = []
    for i in range(8):
        xin = np.empty((NU, B, XR, FW), np.float32)
        bands = np.zeros((128, NU * 4 * M), np.float32)
        for u, (c, k) in enumerate(CORE_UNITS[i][:NU]):
            xbase, _o = _unit_rows(k)
            r = xbase + 4  # padded row index
            for b in range(B):
                xin[u, b] = pad[b, c, r:r + XR, :]
            for t, bb in enumerate(band_cache[k]):
                bands[:, (u * 4 + t) * M:(u * 4 + t + 1) * M] = bb
        in_maps.append({"xin": xin, "bands": bands})

    key = "nc"
    if key not in _NC_CACHE:
        _NC_CACHE[key] = build_nc()
    nc = _NC_CACHE[key]
    r = run_bass_kernel_spmd(nc, in_maps, list(range(8)))
    globals()["LAST_RESULT"] = r
    res = r.results

    out = np.zeros((B, C, H_IMG, W), np.float32)
    for i in range(8):
        for u, (c, k) in enumerate(CORE_UNITS[i][:NU]):
            if i >= 3 and u == 3:
                continue  # dummy repeat
            _xb, out0 = _unit_rows(k)
            rows = 120 if k < 8 else 64
            out[:, c, out0:out0 + rows, :] = \
                res[i]["out"][u, :, :rows, :].astype(np.float32)
    mn, mx = out.min(), out.max()
    return ((out - mn) / (mx - mn)).astype(np.float32)


# revision 18
# speedup vs baseline: 1.0776x; 1.0776x over previous
"""Canny edge detector (nn_CannyNet) on 8 Trainium2 NeuronCores.

Self-contained: hardcodes shapes [4,3,1024,1024] and the filter constants.

Decomposition: 27 units = (3 channels) x (9 row-blocks: 8x120 + 1x64 rows).
Uniform SPMD program: every core processes 4 units (5 cores repeat their
first unit; host ignores the duplicate). Per unit, the 4 batch planes of one
channel are processed together because the reference's flat NMS gather
couples batches.

Engine split (per plane, [<=128, 1024] tiles):
  Pool : s1/s2 col shift-adds, u=v+s1, H=u+w, m2=sqx+sqy (plain TT adds only)
  DVE  : v=s2*R0, w=x*R2 (fast TS), masks + fp16 NMS compare cascade
  PE   : gx/gy directly via accumulating banded matmuls with col-shifted
         rhs views of the zero-padded H tile (vertical conv x horizontal taps)
  Act  : squares (with tan-scale folding) + signs straight from PSUM, fp16 out
All NMS compares run in fp16 (2x DVE rate), planes packed in pairs (1,3)/(0,2)
so one compare instruction covers both planes of an AND-candidate.
Row-shifted m2 views (m2p/m2m) via SBUF->SBUF DMA (engines cannot shift
partitions). Final out is fp16 0/1, converted on host.
"""
import math
import os
import numpy as np

import concourse.bass as bass
import concourse.mybir as mybir
from concourse.bass_utils import run_bass_kernel_spmd

ALU = mybir.AluOpType
AF = mybir.ActivationFunctionType
DT = mybir.dt.float32
F16 = mybir.dt.float16
U16 = mybir.dt.uint16

B, C, H_IMG, W = 4, 3, 1024, 1024
NU = 1 if os.environ.get('KDBG') else 4  # units per core (uniform)
M = 122           # m2/out row span per unit (out 120 + 2)
XR = 128          # x-tile rows
FW = 1028         # x-tile width (cols -2..1025)
HW2 = 1026        # H tile width (cols -1..1024, zero side cols)
MW = 1026         # m2 width per plane (cols -1..1024)

_g = np.exp(-0.5 * np.arange(-2, 3, dtype=np.float64) ** 2)
G1 = _g[1]
R0 = float(np.float32(_g[0] / _g[1]))   # g0/g1
R2 = float(np.float32(1.0 / _g[1]))     # 1/g1
THR = float(np.float32((400.0 / (127.5 * G1)) ** 2))
_t1 = math.tan(22.5 * 3.14159 / 180.0)
_t3 = math.tan(67.5 * 3.14159 / 180.0)

# units and core assignment
UNITS = [(c, k) for k in range(9) for c in range(3)]  # 27
CORE_UNITS = []
for i in range(8):
    us = [UNITS[i], UNITS[i + 8], UNITS[i + 16]]
    us.append(UNITS[24 + i] if i < 3 else UNITS[i])  # dummy repeat for cores 3..7
    CORE_UNITS.append(us)


def _unit_rows(k):
    """(xbase, out0): x-tile img rows xbase..xbase+127; out rows out0..out0+119
    (k=8: only first 64 valid)."""
    if k < 8:
        return 120 * k - 4, 120 * k
    return 900, 960


def _make_bands():
    """CX = S121 @ G, CY = S101 @ G over image rows with zero-pad truncation."""
    n = H_IMG
    G = np.zeros((n, n), np.float64)
    for kk in range(-2, 3):
        v = _g[kk + 2]
        for o in range(max(0, -kk), min(n, n - kk)):
            G[o, o + kk] = v
    S121 = np.zeros((n, n), np.float64)
    S101 = np.zeros((n, n), np.float64)
    for o in range(n):
        for kk, w1, w2 in ((-1, 1.0, 1.0), (0, 2.0, 0.0), (1, 1.0, -1.0)):
            i = o + kk
            if 0 <= i < n:
                S121[o, i] = w1
                if kk != 0:
                    S101[o, i] = w2
    CX = (S121 @ G).astype(np.float32)
    CY = (S101 @ G).astype(np.float32)
    return CX, CY


def _band_lhsT(Cm, k):
    """lhsT [XR, M]: lhsT[kr, m] = Cm[out0-1+m, xbase+kr] (0 out of range)."""
    xbase, out0 = _unit_rows(k)
    out = np.zeros((XR, M), np.float32)
    for m in range(M):
        orow = out0 - 1 + m
        if not (0 <= orow < H_IMG):
            continue
        for d in range(-3, 4):
            irow = orow + d
            kr = irow - xbase
            if 0 <= irow < H_IMG and 0 <= kr < XR:
                out[kr, m] = Cm[orow, irow]
    return out


def build_nc():
    nc = bass.Bass()
    xin = nc.declare_dram_parameter("xin", [NU, B, XR, FW], DT, isOutput=False)
    bands = nc.declare_dram_parameter("bands", [128, NU * 4 * M], DT, isOutput=False)
    outd = nc.declare_dram_parameter("out", [NU, B, 120, W], F16, isOutput=True)
    dbg = None
    if os.environ.get('KDBG'):
        dbg = {
            "dbg_t02": nc.declare_dram_parameter("dbg_t02", [M, 2, MW], DT, isOutput=True),
            "dbg_t13": nc.declare_dram_parameter("dbg_t13", [M, 2, MW], DT, isOutput=True),
            "dbg_mhv": nc.declare_dram_parameter("dbg_mhv", [B, M, 2 * W], U16, isOutput=True),
            "dbg_d1m": nc.declare_dram_parameter("dbg_d1m", [B, M, W], U16, isOutput=True),
            "dbg_h": nc.declare_dram_parameter("dbg_h", [2, XR, HW2], DT, isOutput=True),
            "dbg_sq": nc.declare_dram_parameter("dbg_sq", [2, M, 3 * W], F16, isOutput=True),
        }

    from contextlib import ExitStack
    es = ExitStack()
    ent = es.enter_context

    x = [ent(nc.sbuf_tensor(f"x{b}", [XR, FW], DT)) for b in range(B)]
    s1 = [ent(nc.sbuf_tensor(f"s1_{j}", [XR, W], DT)) for j in range(2)]
    s2 = [ent(nc.sbuf_tensor(f"s2_{j}", [XR, W], DT)) for j in range(2)]
    vv = [ent(nc.sbuf_tensor(f"vv{j}", [XR, W], DT)) for j in range(2)]
    ww = [ent(nc.sbuf_tensor(f"ww{j}", [XR, W], DT)) for j in range(2)]
    Ht = [ent(nc.sbuf_tensor(f"Ht{j}", [XR, HW2], DT)) for j in range(2)]
    ut = s2  # ut aliases s2: s2[j] is dead once DVE v(b) = s2*R0 has read it
    # fp16 working set
    sqa = [ent(nc.sbuf_tensor(f"sqa{j}", [M, 3 * W], F16)) for j in range(2)]
    tsqx = [ent(nc.sbuf_tensor(f"tsqx{j}", [M, W], DT)) for j in range(2)]
    tsqy = [ent(nc.sbuf_tensor(f"tsqy{j}", [M, W], DT)) for j in range(2)]
    sgx = [ent(nc.sbuf_tensor(f"sgx{j}", [M, W], F16)) for j in range(2)]
    sgy = [ent(nc.sbuf_tensor(f"sgy{j}", [M, W], F16)) for j in range(2)]
    mhv = [ent(nc.sbuf_tensor(f"mhv{b}", [M, 2 * W], U16)) for b in range(B)]
    d1m = [ent(nc.sbuf_tensor(f"d1m{b}", [M, W], U16)) for b in range(B)]
    # packed m2 tiles: [M, 2, MW] — slice 0/1 = planes (0,2) in t02, (1,3) in t13
    t02 = ent(nc.sbuf_tensor("t02", [M, 2, MW], DT))
    t13 = ent(nc.sbuf_tensor("t13", [M, 2, MW], DT))
    t02p = ent(nc.sbuf_tensor("t02p", [M, 2, MW], DT))
    t02m = ent(nc.sbuf_tensor("t02m", [M, 2, MW], DT))
    t13p = ent(nc.sbuf_tensor("t13p", [M, 2, MW], DT))
    t13m = ent(nc.sbuf_tensor("t13m", [M, 2, MW], DT))
    cn13 = ent(nc.sbuf_tensor("cn13", [M, 2, W], F16))
    cn02 = ent(nc.sbuf_tensor("cn02", [M, 2, W], F16))
    cp13 = ent(nc.sbuf_tensor("cp13", [M, 2, W], F16))
    cp02 = ent(nc.sbuf_tensor("cp02", [M, 2, W], F16))
    s12 = ent(nc.sbuf_tensor("s12", [M, W], F16))
    aD = ent(nc.sbuf_tensor("aD", [M, W], F16))
    aV = ent(nc.sbuf_tensor("aV", [M, W], F16))
    aH = ent(nc.sbuf_tensor("aH", [M, W], F16))
    thr_t = ent(nc.sbuf_tensor("thr_t", [M, W], F16))
    out_t = [ent(nc.sbuf_tensor(f"out_t{j}", [M, W], F16)) for j in range(2)]
    bnd = ent(nc.sbuf_tensor("bnd", [128, NU * 4 * M], DT))
    pgx = [ent(nc.psum_tensor(f"pgx{j}", [M, W], DT)) for j in range(2)]
    pgy = [ent(nc.psum_tensor(f"pgy{j}", [M, W], DT)) for j in range(2)]

    d_b = ent(nc.semaphore("d_b"))
    d_x = ent(nc.semaphore("d_x"))
    d_sh = ent(nc.semaphore("d_sh"))
    d_out = ent(nc.semaphore("d_out"))
    g_s = ent(nc.semaphore("g_s"))
    g_u = ent(nc.semaphore("g_u"))
    g_h = ent(nc.semaphore("g_h"))
    g_m2 = ent(nc.semaphore("g_m2"))
    v_v = ent(nc.semaphore("v_v"))
    v_w = ent(nc.semaphore("v_w"))
    v_mhv = ent(nc.semaphore("v_mhv"))
    v_d1 = ent(nc.semaphore("v_d1"))
    v_fin = ent(nc.semaphore("v_fin"))
    v_cmp = ent(nc.semaphore("v_cmp"))
    a_sq = ent(nc.semaphore("a_sq"))
    a_sg = ent(nc.semaphore("a_sg"))
    pe = ent(nc.semaphore("pe"))
    block = ent(nc.Block())

    def IX(u, b):
        return 4 * u + b + 1  # 1-based cumulative count at completion of (u,b)

    @block.sync
    def _(sync):
        sync.dma_start(out=bnd[:], in_=bands[:]).then_inc(d_b, 16)
        for b in range(B):
            sync.dma_start(out=x[b][:], in_=xin[0, b]).then_inc(d_x, 16)
        for u in range(NU):
            # x loads for u+1 BEFORE shifts/outs of u: DVE phaseC(u) comes
            # after phaseA(u+1), which needs Pool phase1(u+1) <- these loads.
            if u + 1 < NU:
                for b in range(B):
                    # x[b] WAR: Pool s2 and DVE w of (u,b) read x[b]
                    sync.wait_ge(g_s, IX(u, b))
                    sync.wait_ge(v_w, IX(u, b))
                    sync.dma_start(out=x[b][:], in_=xin[u + 1, b]).then_inc(d_x, 16)
            # per-plane m2 row shifts, launched as soon as each slice lands
            if u > 0:
                sync.wait_ge(v_cmp, 4 * u)  # WAR: NMS cmps of u-1 read shift tiles
            for b in range(B):
                sync.wait_ge(g_m2, IX(u, b))
                src = t02 if b % 2 == 0 else t13
                tp = t02p if b % 2 == 0 else t13p
                tm = t02m if b % 2 == 0 else t13m
                q = b // 2
                sync.dma_start(out=tp[0:M - 1, q], in_=src[1:M, q]).then_inc(d_sh, 16)
                sync.dma_start(out=tm[1:M, q], in_=src[0:M - 1, q]).then_inc(d_sh, 16)
            for b in range(B):
                sync.wait_ge(v_fin, IX(u, b))
                sync.dma_start(out=outd[u, b],
                               in_=out_t[(4 * u + b) % 2][1:121, :]).then_inc(d_out, 16)
        ndbg = 0
        if dbg is not None:
            sync.wait_ge(v_fin, NU * B)
            sync.dma_start(out=dbg["dbg_t02"][:], in_=t02[:]).then_inc(d_out, 16)
            sync.dma_start(out=dbg["dbg_t13"][:], in_=t13[:]).then_inc(d_out, 16)
            for b in range(B):
                sync.dma_start(out=dbg["dbg_mhv"][b], in_=mhv[b][:]).then_inc(d_out, 16)
                sync.dma_start(out=dbg["dbg_d1m"][b], in_=d1m[b][:]).then_inc(d_out, 16)
            for j in range(2):
                sync.dma_start(out=dbg["dbg_h"][j], in_=Ht[j][:]).then_inc(d_out, 16)
                sync.dma_start(out=dbg["dbg_sq"][j], in_=sqa[j][:]).then_inc(d_out, 16)
            ndbg = 14
        sync.wait_ge(d_out, 16 * (NU * B + ndbg))

    @block.gpsimd
    def _(gpsimd):
        # prologue: zero pads once — H side cols; m2 pad cols; shift edge rows
        for j in range(2):
            gpsimd.memset(Ht[j][:, 0:1], 0.0)
            gpsimd.memset(Ht[j][:, 1025:1026], 0.0)
        for t in (t02, t13):
            gpsimd.memset(t[:, :, 0:1], 0.0)
            gpsimd.memset(t[:, :, 1025:1026], 0.0)
        for t in (t02p, t02m, t13p, t13m):
            gpsimd.memset(t[:, :, 0:1], 0.0)
            gpsimd.memset(t[:, :, 1025:1026], 0.0)
        def ph12(u):
            # returns (s1s2, uh) closures; caller interleaves them
            def s1s2(b):
                ix = IX(u, b)
                gpsimd.wait_ge(d_x, 16 * ix)
                if 4 * u + b >= 2:
                    gpsimd.wait_ge(v_v, ix - 2)  # s2 slot WAR (DVE v read)
                gpsimd.tensor_tensor(out=s1[b % 2][:], in0=x[b][:, 1:1025],
                                     in1=x[b][:, 3:1027], op=ALU.add)
                gpsimd.tensor_tensor(out=s2[b % 2][:], in0=x[b][:, 0:1024],
                                     in1=x[b][:, 4:1028],
                                     op=ALU.add).then_inc(g_s, 1)

            def uh(b):
                ix = IX(u, b)
                gpsimd.wait_ge(v_v, ix)
                gpsimd.tensor_tensor(out=ut[b % 2][:], in0=vv[b % 2][:],
                                     in1=s1[b % 2][:], op=ALU.add).then_inc(g_u, 1)
                gpsimd.wait_ge(v_w, ix)
                if 4 * u + b >= 2:
                    gpsimd.wait_ge(pe, ix - 2)  # Ht slot WAR (PE read)
                gpsimd.tensor_tensor(out=Ht[b % 2][:, 1:1025], in0=ut[b % 2][:],
                                     in1=ww[b % 2][:], op=ALU.add).then_inc(g_h, 1)

            return s1s2, uh

        def m2one(u, b):
            ix = IX(u, b)
            gpsimd.wait_ge(a_sq, ix)
            if u > 0 and b == 0:
                gpsimd.wait_ge(v_cmp, 4 * u)   # m2 tiles WAR (NMS cmps of u-1)
                gpsimd.wait_ge(d_sh, 128 * u)  # and shift DMAs of u-1
            tile = t02 if b % 2 == 0 else t13
            q = b // 2
            gpsimd.tensor_tensor(out=tile[:, q, 1:1025], in0=tsqx[b % 2][:],
                                 in1=tsqy[b % 2][:],
                                 op=ALU.add).then_inc(g_m2, 1)

        for u in range(NU):
            sq, uq = ph12(u)
            sq(0); sq(1); uq(0); sq(2); uq(1); sq(3); uq(2); uq(3)
            for b in range(B):
                m2one(u, b)

    @block.vector
    def _(vector):
        def phaseB(u):
            for b in range(B):
                ix = IX(u, b)
                vector.wait_ge(a_sq, ix)
                vector.tensor_tensor(out=mhv[b][:], in0=sqa[b % 2][:, 0:2048],
                                     in1=sqa[b % 2][:, 1024:3072],
                                     op=ALU.is_ge).then_inc(v_mhv, 1)
                vector.wait_ge(a_sg, ix)
                vector.tensor_tensor(out=d1m[b][:], in0=sgx[b % 2][:],
                                     in1=sgy[b % 2][:],
                                     op=ALU.is_equal).then_inc(v_d1, 1)

        def phaseC(u):
            # b=0 (E/W) needs no row-shift tiles: only all m2 slices written
            vector.wait_ge(g_m2, 4 * (u + 1))
            for b in range(B):
                ix = IX(u, b)
                if b == 1:
                    vector.wait_ge(d_sh, 32 * B * (u + 1))
                if b == 0:
                    pv, mv_ = (t02[:, :, 2:1026], t13[:, :, 2:1026]), \
                              (t02[:, :, 0:1024], t13[:, :, 0:1024])
                elif b == 1:
                    pv, mv_ = (t02p[:, :, 2:1026], t13p[:, :, 2:1026]), \
                              (t02m[:, :, 0:1024], t13m[:, :, 0:1024])
                elif b == 2:
                    pv, mv_ = (t02p[:, :, 1:1025], t13p[:, :, 1:1025]), \
                              (t02m[:, :, 1:1025], t13m[:, :, 1:1025])
                else:
                    pv, mv_ = (t02p[:, :, 0:1024], t13p[:, :, 0:1024]), \
                              (t02m[:, :, 2:1026], t13m[:, :, 2:1026])
                c02 = t02[:, :, 1:1025]
                c13 = t13[:, :, 1:1025]
                vector.tensor_tensor(out=cn13[:], in0=c13, in1=mv_[1], op=ALU.is_gt)
                vector.tensor_tensor(out=cn02[:], in0=c02, in1=mv_[0], op=ALU.is_gt)
                vector.tensor_tensor(out=cp13[:], in0=c13, in1=pv[1], op=ALU.is_gt)
                vector.tensor_tensor(out=cp02[:], in0=c02, in1=pv[0], op=ALU.is_gt)
                tile = t02 if b % 2 == 0 else t13
                q = b // 2
                vector.tensor_scalar(out=thr_t[:], in0=tile[:, q, 1:1025],
                                     scalar1=THR, scalar2=None,
                                     op0=ALU.is_ge).then_inc(v_cmp, 1)
                # candidates: default=(c1>m1)&(c3>m3); d1=(c0>m0)&(c2>m2);
                #             mv=(c1>p1)&(c3>p3);      mh=(c0>p0)&(c2>p2)
                vector.tensor_tensor(out=s12[:], in0=cn13[:, 0, :],
                                     in1=cn13[:, 1, :], op=ALU.mult)
                vector.tensor_tensor(out=aD[:], in0=cn02[:, 0, :],
                                     in1=cn02[:, 1, :], op=ALU.mult)
                vector.tensor_tensor(out=aV[:], in0=cp13[:, 0, :],
                                     in1=cp13[:, 1, :], op=ALU.mult)
                vector.tensor_tensor(out=aH[:], in0=cp02[:, 0, :],
                                     in1=cp02[:, 1, :], op=ALU.mult)
                vector.copy_predicated(out=s12[:], mask=d1m[b][:], data=aD[:])
                vector.copy_predicated(out=s12[:], mask=mhv[b][:, 1024:2048],
                                       data=aV[:])
                vector.copy_predicated(out=s12[:], mask=mhv[b][:, 0:1024],
                                       data=aH[:])
                if 4 * u + b >= 2:
                    vector.wait_ge(d_out, 16 * (ix - 2))
                vector.tensor_tensor(out=out_t[(4 * u + b) % 2][:], in0=thr_t[:],
                                     in1=s12[:], op=ALU.mult).then_inc(v_fin, 1)

        for u in range(NU):
            phaseB(u)
            phaseC(u)

    @block.scalar
    def _(scalar):
        def vw(u, b):
            ix = IX(u, b)
            j = b % 2
            scalar.wait_ge(d_x, 16 * ix)
            if 4 * u + b >= 2:
                scalar.wait_ge(g_h, ix - 2)  # ww slot WAR (Pool H read)
            nc.scalar.activation(out=ww[j][:], in_=x[b][:, 2:1026],
                                 func=AF.Copy, scale=R2).then_inc(v_w, 1)
            scalar.wait_ge(g_s, ix)
            if 4 * u + b >= 2:
                scalar.wait_ge(g_u, ix - 2)  # vv slot WAR (Pool ut read)
            nc.scalar.activation(out=vv[j][:], in_=s2[j][:],
                                 func=AF.Copy, scale=R0).then_inc(v_v, 1)

        for u in range(NU):
            for b in range(B):
                vw(u, b)
            for b in range(B):
                ix = IX(u, b)
                scalar.wait_ge(pe, ix)
                if 4 * u + b >= 2:
                    scalar.wait_ge(v_mhv, ix - 2)
                    scalar.wait_ge(g_m2, ix - 2)
                    scalar.wait_ge(v_d1, ix - 2)
                j = b % 2
                p, q = pgx[j], pgy[j]
                nc.scalar.activation(out=sqa[j][:, 0:1024], in_=p[:],
                                     func=AF.Square, scale=_t1)
                nc.scalar.activation(out=sqa[j][:, 1024:2048], in_=q[:],
                                     func=AF.Square)
                nc.scalar.activation(out=sqa[j][:, 2048:3072], in_=p[:],
                                     func=AF.Square, scale=_t3)
                nc.scalar.activation(out=tsqx[j][:], in_=p[:],
                                     func=AF.Square)
                nc.scalar.activation(out=tsqy[j][:], in_=q[:],
                                     func=AF.Square).then_inc(a_sq, 1)
                nc.scalar.activation(out=sgx[j][:], in_=p[:], func=AF.Sign)
                nc.scalar.activation(out=sgy[j][:], in_=q[:],
                                     func=AF.Sign).then_inc(a_sg, 1)

    @block.tensor
    def _(tensor):
        tensor.wait_ge(d_b, 16)
        for u in range(NU):
            off = u * 4 * M
            bxp = bnd[:, off + 0 * M:off + 1 * M]
            bxn = bnd[:, off + 1 * M:off + 2 * M]
            by1 = bnd[:, off + 2 * M:off + 3 * M]
            by2 = bnd[:, off + 3 * M:off + 4 * M]
            for b in range(B):
                ix = IX(u, b)
                tensor.wait_ge(g_h, ix)
                if 4 * u + b >= 2:
                    tensor.wait_ge(a_sg, ix - 2)  # PSUM pair WAR (Act reads)
                j = b % 2
                p, q, H_ = pgx[j], pgy[j], Ht[j]
                mm = nc.tensor.matmul
                mm(p[:, 0:512], bxp, H_[:, 0:512], start=True, stop=False)
                mm(p[:, 512:1024], bxp, H_[:, 512:1024], start=True, stop=False)
                mm(p[:, 0:512], bxn, H_[:, 2:514], start=False, stop=True)
                mm(p[:, 512:1024], bxn, H_[:, 514:1026], start=False, stop=True)
                mm(q[:, 0:512], by1, H_[:, 0:512], start=True, stop=False)
                mm(q[:, 512:1024], by1, H_[:, 512:1024], start=True, stop=False)
                mm(q[:, 0:512], by1, H_[:, 2:514], start=False, stop=False)
                mm(q[:, 512:1024], by1, H_[:, 514:1026], start=False, stop=False)
                mm(q[:, 0:512], by2, H_[:, 1:513], start=False, stop=True)
                mm(q[:, 512:1024], by2, H_[:, 513:1025], start=False,
                   stop=True).then_inc(pe, 1)

    es.close()
    return nc


_NC_CACHE = {}


def kernel(img, gauss_h=None, gauss_v=None, sobel_h=None, sobel_v=None,
           dir_w=None, **_):
    img = np.asarray(img, dtype=np.float32)
    assert img.shape == (B, C, H_IMG, W)

    # host pad with 0 and fold the (+1) affine in
    pad = np.zeros((B, C, H_IMG + 8, W + 4), np.float32)
    pad[:, :, 4:4 + H_IMG, 2:2 + W] = img + 1.0

    CX, CY = _make_bands()
    band_cache = {}
    for c, k in UNITS:
        if k not in band_cache:
            bx = _band_lhsT(CX, k)
            by = _band_lhsT(CY, k)
            band_cache[k] = (bx, -bx, by, 2.0 * by)

    in_maps = []
    for i in range(8):
        xin = np.empty((NU, B, XR, FW), np.float32)
        bands = np.zeros((128, NU * 4 * M), np.float32)
        for u, (c, k) in enumerate(CORE_UNITS[i][:NU]):
            xbase, _o = _unit_rows(k)
            r = xbase + 4  # padded row index
            for b in range(B):
                xin[u, b] = pad[b, c, r:r + XR, :]
            for t, bb in enumerate(band_cache[k]):
                bands[:, (u * 4 + t) * M:(u * 4 + t + 1) * M] = bb
        in_maps.append({"xin": xin, "bands": bands})

    key = "nc"
    if key not in _NC_CACHE:
        _NC_CACHE[key] = build_nc()
    nc = _NC_CACHE[key]
    r = run_bass_kernel_spmd(nc, in_maps, list(range(8)))
    globals()["LAST_RESULT"] = r
    res = r.results

    out = np.zeros((B, C, H_IMG, W), np.float32)
    for i in range(8):
        for u, (c, k) in enumerate(CORE_UNITS[i][:NU]):
            if i >= 3 and u == 3:
                continue  # dummy repeat
            _xb, out0 = _unit_rows(k)
            rows = 120 if k < 8 else 64
            out[:, c, out0:out0 + rows, :] = \
                res[i]["out"][u, :, :rows, :].astype(np.float32)
    mn, mx = out.min(), out.max()
    return ((out - mn) / (mx - mn)).astype(np.float32)


# revision 19
# speedup vs baseline: 1.0785x; 1.0008x over previous
"""Canny edge detector (nn_CannyNet) on 8 Trainium2 NeuronCores.

Self-contained: hardcodes shapes [4,3,1024,1024] and the filter constants.

Decomposition: 27 units = (3 channels) x (9 row-blocks: 8x120 + 1x64 rows).
Uniform SPMD program: every core processes 4 units (5 cores repeat their
first unit; host ignores the duplicate). Per unit, the 4 batch planes of one
channel are processed together because the reference's flat NMS gather
couples batches.

Engine split (per plane, [<=128, 1024] tiles):
  Pool : s1/s2 col shift-adds, u=v+s1, H=u+w, m2=sqx+sqy (plain TT adds only;
         tensor_scalar on Pool is ~7x slower than roofline - never use it)
  Act  : v=s2*R0, w=x*R2 (Copy+scale), squares (tan-scale folded) + signs
         read directly from PSUM; masks go out fp16, m2 inputs fp32
  PE   : gx/gy directly via accumulating banded matmuls with col-shifted
         rhs views of the zero-padded H tile (vertical conv x horizontal taps)
  DVE  : orientation masks (fp16, 2x rate) + NMS compare cascade. m2 and its
         neighbor/threshold compares MUST be fp32: fp16 m2 costs ~1550 output
         flips (near-tie neighbor compares), vs 2e-2 rel-err budget ~1070.
Planes are packed in pairs (1,3)/(0,2) in [M,2,1026] tiles so one compare
instruction covers both planes of an AND-candidate. Row-shifted m2 views
(m2p/m2m) via SBUF->SBUF DMA (compute ops require equal base partitions).
Final out is fp16 0/1, converted on host. The (+1) affine is folded into the
host pad buffer (in-place tensor_scalar on Pool measured 14.7us/tile).
"""
import math
import os
import numpy as np

import concourse.bass as bass
import concourse.mybir as mybir
from concourse.bass_utils import run_bass_kernel_spmd

ALU = mybir.AluOpType
AF = mybir.ActivationFunctionType
DT = mybir.dt.float32
F16 = mybir.dt.float16
U16 = mybir.dt.uint16

B, C, H_IMG, W = 4, 3, 1024, 1024
NU = 1 if os.environ.get('KDBG') else 4  # units per core (uniform)
M = 122           # m2/out row span per unit (out 120 + 2)
XR = 128          # x-tile rows
FW = 1028         # x-tile width (cols -2..1025)
HW2 = 1026        # H tile width (cols -1..1024, zero side cols)
MW = 1026         # m2 width per plane (cols -1..1024)

_g = np.exp(-0.5 * np.arange(-2, 3, dtype=np.float64) ** 2)
G1 = _g[1]
R0 = float(np.float32(_g[0] / _g[1]))   # g0/g1
R2 = float(np.float32(1.0 / _g[1]))     # 1/g1
THR = float(np.float32((400.0 / (127.5 * G1)) ** 2))
_t1 = math.tan(22.5 * 3.14159 / 180.0)
_t3 = math.tan(67.5 * 3.14159 / 180.0)

# units and core assignment
UNITS = [(c, k) for k in range(9) for c in range(3)]  # 27
CORE_UNITS = []
for i in range(8):
    us = [UNITS[i], UNITS[i + 8], UNITS[i + 16]]
    us.append(UNITS[24 + i] if i < 3 else UNITS[i])  # dummy repeat for cores 3..7
    CORE_UNITS.append(us)


def _unit_rows(k):
    """(xbase, out0): x-tile img rows xbase..xbase+127; out rows out0..out0+119
    (k=8: only first 64 valid)."""
    if k < 8:
        return 120 * k - 4, 120 * k
    return 900, 960


def _make_bands():
    """CX = S121 @ G, CY = S101 @ G over image rows with zero-pad truncation."""
    n = H_IMG
    G = np.zeros((n, n), np.float64)
    for kk in range(-2, 3):
        v = _g[kk + 2]
        for o in range(max(0, -kk), min(n, n - kk)):
            G[o, o + kk] = v
    S121 = np.zeros((n, n), np.float64)
    S101 = np.zeros((n, n), np.float64)
    for o in range(n):
        for kk, w1, w2 in ((-1, 1.0, 1.0), (0, 2.0, 0.0), (1, 1.0, -1.0)):
            i = o + kk
            if 0 <= i < n:
                S121[o, i] = w1
                if kk != 0:
                    S101[o, i] = w2
    CX = (S121 @ G).astype(np.float32)
    CY = (S101 @ G).astype(np.float32)
    return CX, CY


def _band_lhsT(Cm, k):
    """lhsT [XR, M]: lhsT[kr, m] = Cm[out0-1+m, xbase+kr] (0 out of range)."""
    xbase, out0 = _unit_rows(k)
    out = np.zeros((XR, M), np.float32)
    for m in range(M):
        orow = out0 - 1 + m
        if not (0 <= orow < H_IMG):
            continue
        for d in range(-3, 4):
            irow = orow + d
            kr = irow - xbase
            if 0 <= irow < H_IMG and 0 <= kr < XR:
                out[kr, m] = Cm[orow, irow]
    return out


def build_nc():
    nc = bass.Bass()
    xin = nc.declare_dram_parameter("xin", [NU, B, XR, FW], DT, isOutput=False)
    bands = nc.declare_dram_parameter("bands", [128, NU * 4 * M], DT, isOutput=False)
    outd = nc.declare_dram_parameter("out", [NU, B, 120, W], F16, isOutput=True)
    dbg = None
    if os.environ.get('KDBG'):
        dbg = {
            "dbg_t02": nc.declare_dram_parameter("dbg_t02", [M, 2, MW], DT, isOutput=True),
            "dbg_t13": nc.declare_dram_parameter("dbg_t13", [M, 2, MW], DT, isOutput=True),
            "dbg_mhv": nc.declare_dram_parameter("dbg_mhv", [B, M, 2 * W], U16, isOutput=True),
            "dbg_d1m": nc.declare_dram_parameter("dbg_d1m", [B, M, W], U16, isOutput=True),
            "dbg_h": nc.declare_dram_parameter("dbg_h", [2, XR, HW2], DT, isOutput=True),
            "dbg_sq": nc.declare_dram_parameter("dbg_sq", [2, M, 3 * W], F16, isOutput=True),
        }

    from contextlib import ExitStack
    es = ExitStack()
    ent = es.enter_context

    x = [ent(nc.sbuf_tensor(f"x{b}", [XR, FW], DT)) for b in range(B)]
    s1 = [ent(nc.sbuf_tensor(f"s1_{j}", [XR, W], DT)) for j in range(2)]
    s2 = [ent(nc.sbuf_tensor(f"s2_{j}", [XR, W], DT)) for j in range(2)]
    vv = [ent(nc.sbuf_tensor(f"vv{j}", [XR, W], DT)) for j in range(2)]
    ww = [ent(nc.sbuf_tensor(f"ww{j}", [XR, W], DT)) for j in range(2)]
    Ht = [ent(nc.sbuf_tensor(f"Ht{j}", [XR, HW2], DT)) for j in range(2)]
    ut = s2  # ut aliases s2: s2[j] is dead once DVE v(b) = s2*R0 has read it
    # fp16 working set
    sqa = [ent(nc.sbuf_tensor(f"sqa{j}", [M, 3 * W], F16)) for j in range(2)]
    tsqx = [ent(nc.sbuf_tensor(f"tsqx{j}", [M, W], DT)) for j in range(2)]
    tsqy = [ent(nc.sbuf_tensor(f"tsqy{j}", [M, W], DT)) for j in range(2)]
    sgx = [ent(nc.sbuf_tensor(f"sgx{j}", [M, W], F16)) for j in range(2)]
    sgy = [ent(nc.sbuf_tensor(f"sgy{j}", [M, W], F16)) for j in range(2)]
    mhv = [ent(nc.sbuf_tensor(f"mhv{b}", [M, 2 * W], U16)) for b in range(B)]
    d1m = [ent(nc.sbuf_tensor(f"d1m{b}", [M, W], U16)) for b in range(B)]
    # packed m2 tiles: [M, 2, MW] — slice 0/1 = planes (0,2) in t02, (1,3) in t13
    t02 = ent(nc.sbuf_tensor("t02", [M, 2, MW], DT))
    t13 = ent(nc.sbuf_tensor("t13", [M, 2, MW], DT))
    t02p = ent(nc.sbuf_tensor("t02p", [M, 2, MW], DT))
    t02m = ent(nc.sbuf_tensor("t02m", [M, 2, MW], DT))
    t13p = ent(nc.sbuf_tensor("t13p", [M, 2, MW], DT))
    t13m = ent(nc.sbuf_tensor("t13m", [M, 2, MW], DT))
    cn13 = ent(nc.sbuf_tensor("cn13", [M, 2, W], F16))
    cn02 = ent(nc.sbuf_tensor("cn02", [M, 2, W], F16))
    cp13 = ent(nc.sbuf_tensor("cp13", [M, 2, W], F16))
    cp02 = ent(nc.sbuf_tensor("cp02", [M, 2, W], F16))
    s12 = ent(nc.sbuf_tensor("s12", [M, W], F16))
    aD = ent(nc.sbuf_tensor("aD", [M, W], F16))
    aV = ent(nc.sbuf_tensor("aV", [M, W], F16))
    aH = ent(nc.sbuf_tensor("aH", [M, W], F16))
    thr_t = ent(nc.sbuf_tensor("thr_t", [M, W], F16))
    out_t = [ent(nc.sbuf_tensor(f"out_t{j}", [M, W], F16)) for j in range(2)]
    bnd = ent(nc.sbuf_tensor("bnd", [128, NU * 4 * M], DT))
    pgx = [ent(nc.psum_tensor(f"pgx{j}", [M, W], DT)) for j in range(2)]
    pgy = [ent(nc.psum_tensor(f"pgy{j}", [M, W], DT)) for j in range(2)]

    d_b = ent(nc.semaphore("d_b"))
    d_x = ent(nc.semaphore("d_x"))
    d_sh = ent(nc.semaphore("d_sh"))
    d_out = ent(nc.semaphore("d_out"))
    g_s = ent(nc.semaphore("g_s"))
    g_u = ent(nc.semaphore("g_u"))
    g_h = ent(nc.semaphore("g_h"))
    g_m2 = ent(nc.semaphore("g_m2"))
    v_v = ent(nc.semaphore("v_v"))
    v_w = ent(nc.semaphore("v_w"))
    v_mhv = ent(nc.semaphore("v_mhv"))
    v_d1 = ent(nc.semaphore("v_d1"))
    v_fin = ent(nc.semaphore("v_fin"))
    v_cmp = ent(nc.semaphore("v_cmp"))
    a_sq = ent(nc.semaphore("a_sq"))
    a_sg = ent(nc.semaphore("a_sg"))
    pe = ent(nc.semaphore("pe"))
    block = ent(nc.Block())

    def IX(u, b):
        return 4 * u + b + 1  # 1-based cumulative count at completion of (u,b)

    @block.sync
    def _(sync):
        sync.dma_start(out=bnd[:], in_=bands[:]).then_inc(d_b, 16)
        for b in range(B):
            sync.dma_start(out=x[b][:], in_=xin[0, b]).then_inc(d_x, 16)
        for u in range(NU):
            # x loads for u+1 BEFORE shifts/outs of u: DVE phaseC(u) comes
            # after phaseA(u+1), which needs Pool phase1(u+1) <- these loads.
            if u + 1 < NU:
                for b in range(B):
                    # x[b] WAR: Pool s2 and DVE w of (u,b) read x[b]
                    sync.wait_ge(g_s, IX(u, b))
                    sync.wait_ge(v_w, IX(u, b))
                    sync.dma_start(out=x[b][:], in_=xin[u + 1, b]).then_inc(d_x, 16)
            # per-plane m2 row shifts, launched as soon as each slice lands
            if u > 0:
                sync.wait_ge(v_cmp, 4 * u)  # WAR: NMS cmps of u-1 read shift tiles
            for b in range(B):
                sync.wait_ge(g_m2, IX(u, b))
                src = t02 if b % 2 == 0 else t13
                tp = t02p if b % 2 == 0 else t13p
                tm = t02m if b % 2 == 0 else t13m
                q = b // 2
                sync.dma_start(out=tp[0:M - 1, q], in_=src[1:M, q]).then_inc(d_sh, 16)
                sync.dma_start(out=tm[1:M, q], in_=src[0:M - 1, q]).then_inc(d_sh, 16)
            for b in range(B):
                sync.wait_ge(v_fin, IX(u, b))
                sync.dma_start(out=outd[u, b],
                               in_=out_t[(4 * u + b) % 2][1:121, :]).then_inc(d_out, 16)
        ndbg = 0
        if dbg is not None:
            sync.wait_ge(v_fin, NU * B)
            sync.dma_start(out=dbg["dbg_t02"][:], in_=t02[:]).then_inc(d_out, 16)
            sync.dma_start(out=dbg["dbg_t13"][:], in_=t13[:]).then_inc(d_out, 16)
            for b in range(B):
                sync.dma_start(out=dbg["dbg_mhv"][b], in_=mhv[b][:]).then_inc(d_out, 16)
                sync.dma_start(out=dbg["dbg_d1m"][b], in_=d1m[b][:]).then_inc(d_out, 16)
            for j in range(2):
                sync.dma_start(out=dbg["dbg_h"][j], in_=Ht[j][:]).then_inc(d_out, 16)
                sync.dma_start(out=dbg["dbg_sq"][j], in_=sqa[j][:]).then_inc(d_out, 16)
            ndbg = 14
        sync.wait_ge(d_out, 16 * (NU * B + ndbg))

    @block.gpsimd
    def _(gpsimd):
        # prologue: zero pads once — H side cols; m2 pad cols; shift edge rows
        for j in range(2):
            gpsimd.memset(Ht[j][:, 0:1], 0.0)
            gpsimd.memset(Ht[j][:, 1025:1026], 0.0)
        for t in (t02, t13):
            gpsimd.memset(t[:, :, 0:1], 0.0)
            gpsimd.memset(t[:, :, 1025:1026], 0.0)
        for t in (t02p, t02m, t13p, t13m):
            gpsimd.memset(t[:, :, 0:1], 0.0)
            gpsimd.memset(t[:, :, 1025:1026], 0.0)
        def ph12(u):
            # returns (s1s2, uh) closures; caller interleaves them
            def s1s2(b):
                ix = IX(u, b)
                gpsimd.wait_ge(d_x, 16 * ix)
                if 4 * u + b >= 2:
                    gpsimd.wait_ge(v_v, ix - 2)  # s2 slot WAR (DVE v read)
                gpsimd.tensor_tensor(out=s1[b % 2][:], in0=x[b][:, 1:1025],
                                     in1=x[b][:, 3:1027], op=ALU.add)
                gpsimd.tensor_tensor(out=s2[b % 2][:], in0=x[b][:, 0:1024],
                                     in1=x[b][:, 4:1028],
                                     op=ALU.add).then_inc(g_s, 1)

            def uh(b):
                ix = IX(u, b)
                gpsimd.wait_ge(v_v, ix)
                gpsimd.tensor_tensor(out=ut[b % 2][:], in0=vv[b % 2][:],
                                     in1=s1[b % 2][:], op=ALU.add).then_inc(g_u, 1)
                gpsimd.wait_ge(v_w, ix)
                if 4 * u + b >= 2:
                    gpsimd.wait_ge(pe, ix - 2)  # Ht slot WAR (PE read)
                gpsimd.tensor_tensor(out=Ht[b % 2][:, 1:1025], in0=ut[b % 2][:],
                                     in1=ww[b % 2][:], op=ALU.add).then_inc(g_h, 1)

            return s1s2, uh

        def m2one(u, b):
            ix = IX(u, b)
            gpsimd.wait_ge(a_sq, ix)
            if u > 0 and b == 0:
                gpsimd.wait_ge(v_cmp, 4 * u)   # m2 tiles WAR (NMS cmps of u-1)
                gpsimd.wait_ge(d_sh, 128 * u)  # and shift DMAs of u-1
            tile = t02 if b % 2 == 0 else t13
            q = b // 2
            gpsimd.tensor_tensor(out=tile[:, q, 1:1025], in0=tsqx[b % 2][:],
                                 in1=tsqy[b % 2][:],
                                 op=ALU.add).then_inc(g_m2, 1)

        for u in range(NU):
            sq, uq = ph12(u)
            sq(0); sq(1); uq(0); sq(2); uq(1); sq(3); uq(2); uq(3)
            for b in range(B):
                m2one(u, b)

    @block.vector
    def _(vector):
        def phaseB(u):
            for b in range(B):
                ix = IX(u, b)
                vector.wait_ge(a_sq, ix)
                vector.tensor_tensor(out=mhv[b][:], in0=sqa[b % 2][:, 0:2048],
                                     in1=sqa[b % 2][:, 1024:3072],
                                     op=ALU.is_ge).then_inc(v_mhv, 1)
                vector.wait_ge(a_sg, ix)
                vector.tensor_tensor(out=d1m[b][:], in0=sgx[b % 2][:],
                                     in1=sgy[b % 2][:],
                                     op=ALU.is_equal).then_inc(v_d1, 1)

        def phaseC(u):
            # b=0 (E/W) needs no row-shift tiles: only all m2 slices written
            vector.wait_ge(g_m2, 4 * (u + 1))
            for b in range(B):
                ix = IX(u, b)
                if b == 1:
                    vector.wait_ge(d_sh, 32 * B * (u + 1))
                if b == 0:
                    pv, mv_ = (t02[:, :, 2:1026], t13[:, :, 2:1026]), \
                              (t02[:, :, 0:1024], t13[:, :, 0:1024])
                elif b == 1:
                    pv, mv_ = (t02p[:, :, 2:1026], t13p[:, :, 2:1026]), \
                              (t02m[:, :, 0:1024], t13m[:, :, 0:1024])
                elif b == 2:
                    pv, mv_ = (t02p[:, :, 1:1025], t13p[:, :, 1:1025]), \
                              (t02m[:, :, 1:1025], t13m[:, :, 1:1025])
                else:
                    pv, mv_ = (t02p[:, :, 0:1024], t13p[:, :, 0:1024]), \
                              (t02m[:, :, 2:1026], t13m[:, :, 2:1026])
                c02 = t02[:, :, 1:1025]
                c13 = t13[:, :, 1:1025]
                vector.tensor_tensor(out=cn13[:], in0=c13, in1=mv_[1], op=ALU.is_gt)
                vector.tensor_tensor(out=cn02[:], in0=c02, in1=mv_[0], op=ALU.is_gt)
                vector.tensor_tensor(out=cp13[:], in0=c13, in1=pv[1], op=ALU.is_gt)
                vector.tensor_tensor(out=cp02[:], in0=c02, in1=pv[0], op=ALU.is_gt)
                tile = t02 if b % 2 == 0 else t13
                q = b // 2
                vector.tensor_scalar(out=thr_t[:], in0=tile[:, q, 1:1025],
                                     scalar1=THR, scalar2=None,
                                     op0=ALU.is_ge).then_inc(v_cmp, 1)
                # candidates: default=(c1>m1)&(c3>m3); d1=(c0>m0)&(c2>m2);
                #             mv=(c1>p1)&(c3>p3);      mh=(c0>p0)&(c2>p2)
                vector.tensor_tensor(out=s12[:], in0=cn13[:, 0, :],
                                     in1=cn13[:, 1, :], op=ALU.mult)
                vector.tensor_tensor(out=aD[:], in0=cn02[:, 0, :],
                                     in1=cn02[:, 1, :], op=ALU.mult)
                vector.tensor_tensor(out=aV[:], in0=cp13[:, 0, :],
                                     in1=cp13[:, 1, :], op=ALU.mult)
                vector.tensor_tensor(out=aH[:], in0=cp02[:, 0, :],
                                     in1=cp02[:, 1, :], op=ALU.mult)
                vector.copy_predicated(out=s12[:], mask=d1m[b][:], data=aD[:])
                vector.copy_predicated(out=s12[:], mask=mhv[b][:, 1024:2048],
                                       data=aV[:])
                vector.copy_predicated(out=s12[:], mask=mhv[b][:, 0:1024],
                                       data=aH[:])
                if 4 * u + b >= 2:
                    vector.wait_ge(d_out, 16 * (ix - 2))
                vector.tensor_tensor(out=out_t[(4 * u + b) % 2][:], in0=thr_t[:],
                                     in1=s12[:], op=ALU.mult).then_inc(v_fin, 1)

        for u in range(NU):
            phaseB(u)
            phaseC(u)

    @block.scalar
    def _(scalar):
        def vw(u, b):
            ix = IX(u, b)
            j = b % 2
            scalar.wait_ge(d_x, 16 * ix)
            if 4 * u + b >= 2:
                scalar.wait_ge(g_h, ix - 2)  # ww slot WAR (Pool H read)
            nc.scalar.activation(out=ww[j][:], in_=x[b][:, 2:1026],
                                 func=AF.Copy, scale=R2).then_inc(v_w, 1)
            scalar.wait_ge(g_s, ix)
            if 4 * u + b >= 2:
                scalar.wait_ge(g_u, ix - 2)  # vv slot WAR (Pool ut read)
            nc.scalar.activation(out=vv[j][:], in_=s2[j][:],
                                 func=AF.Copy, scale=R0).then_inc(v_v, 1)

        for u in range(NU):
            for b in range(B):
                vw(u, b)
            for b in range(B):
                ix = IX(u, b)
                scalar.wait_ge(pe, ix)
                if 4 * u + b >= 2:
                    scalar.wait_ge(v_mhv, ix - 2)
                    scalar.wait_ge(g_m2, ix - 2)
                    scalar.wait_ge(v_d1, ix - 2)
                j = b % 2
                p, q = pgx[j], pgy[j]
                nc.scalar.activation(out=sqa[j][:, 0:1024], in_=p[:],
                                     func=AF.Square, scale=_t1)
                nc.scalar.activation(out=sqa[j][:, 1024:2048], in_=q[:],
                                     func=AF.Square)
                nc.scalar.activation(out=sqa[j][:, 2048:3072], in_=p[:],
                                     func=AF.Square, scale=_t3)
                nc.scalar.activation(out=tsqx[j][:], in_=p[:],
                                     func=AF.Square)
                nc.scalar.activation(out=tsqy[j][:], in_=q[:],
                                     func=AF.Square).then_inc(a_sq, 1)
                nc.scalar.activation(out=sgx[j][:], in_=p[:], func=AF.Sign)
                nc.scalar.activation(out=sgy[j][:], in_=q[:],
                                     func=AF.Sign).then_inc(a_sg, 1)

    @block.tensor
    def _(tensor):
        tensor.wait_ge(d_b, 16)
        for u in range(NU):
            off = u * 4 * M
            bxp = bnd[:, off + 0 * M:off + 1 * M]
            bxn = bnd[:, off + 1 * M:off + 2 * M]
            by1 = bnd[:, off + 2 * M:off + 3 * M]
            by2 = bnd[:, off + 3 * M:off + 4 * M]
            for b in range(B):
                ix = IX(u, b)
                tensor.wait_ge(g_h, ix)
                if 4 * u + b >= 2:
                    tensor.wait_ge(a_sg, ix - 2)  # PSUM pair WAR (Act reads)
                j = b % 2
                p, q, H_ = pgx[j], pgy[j], Ht[j]
                mm = nc.tensor.matmul
                mm(p[:, 0:512], bxp, H_[:, 0:512], start=True, stop=False)
                mm(p[:, 512:1024], bxp, H_[:, 512:1024], start=True, stop=False)
                mm(p[:, 0:512], bxn, H_[:, 2:514], start=False, stop=True)
                mm(p[:, 512:1024], bxn, H_[:, 514:1026], start=False, stop=True)
                mm(q[:, 0:512], by1, H_[:, 0:512], start=True, stop=False)
                mm(q[:, 512:1024], by1, H_[:, 512:1024], start=True, stop=False)
                mm(q[:, 0:512], by1, H_[:, 2:514], start=False, stop=False)
                mm(q[:, 512:1024], by1, H_[:, 514:1026], start=False, stop=False)
                mm(q[:, 0:512], by2, H_[:, 1:513], start=False, stop=True)
                mm(q[:, 512:1024], by2, H_[:, 513:1025], start=False,
                   stop=True).then_inc(pe, 1)

    es.close()
    return nc


_NC_CACHE = {}


def kernel(img, gauss_h=None, gauss_v=None, sobel_h=None, sobel_v=None,
           dir_w=None, **_):
    img = np.asarray(img, dtype=np.float32)
    assert img.shape == (B, C, H_IMG, W)

    # host pad with 0 and fold the (+1) affine in
    pad = np.zeros((B, C, H_IMG + 8, W + 4), np.float32)
    pad[:, :, 4:4 + H_IMG, 2:2 + W] = img + 1.0

    CX, CY = _make_bands()
    band_cache = {}
    for c, k in UNITS:
        if k not in band_cache:
            bx = _band_lhsT(CX, k)
            by = _band_lhsT(CY, k)
            band_cache[k] = (bx, -bx, by, 2.0 * by)

    in_maps = []
    for i in range(8):
        xin = np.empty((NU, B, XR, FW), np.float32)
        bands = np.zeros((128, NU * 4 * M), np.float32)
        for u, (c, k) in enumerate(CORE_UNITS[i][:NU]):
            xbase, _o = _unit_rows(k)
            r = xbase + 4  # padded row index
            for b in range(B):
                xin[u, b] = pad[b, c, r:r + XR, :]
            for t, bb in enumerate(band_cache[k]):
                bands[:, (u * 4 + t) * M:(u * 4 + t + 1) * M] = bb
        in_maps.append({"xin": xin, "bands": bands})

    key = "nc"
    if key not in _NC_CACHE:
        _NC_CACHE[key] = build_nc()
    nc = _NC_CACHE[key]
    r = run_bass_kernel_spmd(nc, in_maps, list(range(8)))
    globals()["LAST_RESULT"] = r
    res = r.results

    out = np.zeros((B, C, H_IMG, W), np.float32)
    for i in range(8):
        for u, (c, k) in enumerate(CORE_UNITS[i][:NU]):
            if i >= 3 and u == 3:
                continue  # dummy repeat
            _xb, out0 = _unit_rows(k)
            rows = 120 if k < 8 else 64
            out[:, c, out0:out0 + rows, :] = \
                res[i]["out"][u, :, :rows, :].astype(np.float32)
    mn, mx = out.min(), out.max()
    return ((out - mn) / (mx - mn)).astype(np.float32)
